# revision 8
# baseline (speedup 1.0000x reference)
"""Trainium2 Bass kernel for nn_DivFreeField: divergence-free field from a
3-layer ReLU MLP stream function, computed via forward-mode Jacobian.

Math (D=3, H=256, N=262144):
  coefs c(x) = MLP(x);  A = skew(c);  out_j = sum_i dA_ij/dx_i
  With G[a,k] = dc_a/dx_k:
    out_0 =  G[0,1] + G[1,2]
    out_1 = -G[0,0] + G[2,2]
    out_2 = -G[1,0] - G[2,1]
  Forward-mode: 3 tangent streams alongside the primal. ReLU masks must be
  computed from true-fp32 pre-activations (mask flips near zero dominate the
  error); tangent GEMMs run in f32r (TF32) which is well inside the fp32
  reordering envelope.

Sharding: pure data parallel over N across 8 cores (weights replicated).
"""
import numpy as np

import concourse.bass as bass
import concourse.bacc as bacc
import concourse.tile as tile
import concourse.mybir as mybir
from concourse.bass_utils import run_bass_kernel_spmd

F32 = mybir.dt.float32
F32R = mybir.dt.float32r
RELU = mybir.ActivationFunctionType.Relu
COPY = mybir.ActivationFunctionType.Copy
GT = mybir.AluOpType.is_gt
MUL = mybir.AluOpType.mult

N = 262144
D = 3
H = 256
NCORES = 8
SHARD = N // NCORES      # 32768
T = 512                  # tokens per tile
NT = SHARD // T          # 64


def build_nc(reps: int = 1, hs_bufs: int = 2, us_bufs: int = 4, io_bufs: int = 4):
    nc = bacc.Bacc("TRN2", target_bir_lowering=False, debug=False)

    xt_d = nc.dram_tensor("xt", [D, SHARD], F32, kind="ExternalInput").ap()
    w1_d = nc.dram_tensor("w1", [D, H], F32, kind="ExternalInput").ap()
    w2_d = nc.dram_tensor("w2", [H, H], F32, kind="ExternalInput").ap()
    w3_d = nc.dram_tensor("w3", [H, H], F32, kind="ExternalInput").ap()
    vp_d = nc.dram_tensor("vp", [128, 18], F32, kind="ExternalInput").ap()
    w1r_d = nc.dram_tensor("w1r", [128, 6], F32, kind="ExternalInput").ap()
    bias_d = nc.dram_tensor("bias", [128, 6], F32, kind="ExternalInput").ap()
    yt_d = nc.dram_tensor("yt", [D, SHARD], F32, kind="ExternalOutput").ap()

    with tile.TileContext(nc) as tc:
        with (
            tc.tile_pool(name="wp", bufs=1) as wp,
            tc.tile_pool(name="io", bufs=io_bufs) as io,
            tc.tile_pool(name="hs", bufs=hs_bufs) as hs,
            tc.tile_pool(name="us", bufs=us_bufs) as us,
            tc.tile_pool(name="ps", bufs=8, space="PSUM") as ps,
        ):
            # ---- persistent weights ----
            w1 = wp.tile([D, H], F32, tag="w1", name="w1")
            nc.sync.dma_start(out=w1[:], in_=w1_d)
            w2 = [wp.tile([128, H], F32, tag=f"w2_{k}", name=f"w2_{k}")
                  for k in range(2)]
            w3 = [wp.tile([128, H], F32, tag=f"w3_{k}", name=f"w3_{k}")
                  for k in range(2)]
            for k in range(2):
                nc.sync.dma_start(out=w2[k][:], in_=w2_d[128 * k:128 * (k + 1), :])
                nc.sync.dma_start(out=w3[k][:], in_=w3_d[128 * k:128 * (k + 1), :])
            # f32r copies for tangent GEMMs
            w2r = [wp.tile([128, H], F32R, tag=f"w2r_{k}", name=f"w2r_{k}")
                   for k in range(2)]
            w3r = [wp.tile([128, H], F32R, tag=f"w3r_{k}", name=f"w3r_{k}")
                   for k in range(2)]
            for k in range(2):
                nc.vector.tensor_copy(w2r[k][:], w2[k][:])
                nc.vector.tensor_copy(w3r[k][:], w3[k][:])
            vp_f = wp.tile([128, 18], F32, tag="vp_f", name="vp_f")
            nc.sync.dma_start(out=vp_f[:], in_=vp_d)
            vp = wp.tile([128, 18], F32R, tag="vp", name="vp")
            nc.vector.tensor_copy(vp[:], vp_f[:])
            w1r = wp.tile([128, 6], F32, tag="w1r", name="w1r")
            nc.sync.dma_start(out=w1r[:], in_=w1r_d)
            bia = wp.tile([128, 6], F32, tag="bia", name="bia")
            nc.sync.dma_start(out=bia[:], in_=bias_d)

            loop = tc.For_i(0, reps, 1) if reps > 1 else None
            if loop is not None:
                loop.__enter__()

            for it in range(NT):
                t0 = it * T
                sfx = f"_{it}"

                xT = io.tile([D, T], F32, tag="xT", name="xT" + sfx)
                nc.sync.dma_start(out=xT[:], in_=xt_d[:, t0:t0 + T])

                # ---- L1: z1 = W1^T x  (fp32), h1 = relu(z1+b1),
                #      u1_k = (h1>0) * W1[k,:]
                z1 = [ps.tile([128, T], F32, tag="ps", name=f"z1_{m}{sfx}")
                      for m in range(2)]
                for m in range(2):
                    nc.tensor.matmul(z1[m][:], lhsT=w1[:, bass.ts(m, 128)],
                                     rhs=xT[:], start=True, stop=True)
                h1 = [hs.tile([128, T], F32, tag=f"h1_{m}", name=f"h1_{m}{sfx}")
                      for m in range(2)]
                for m in range(2):
                    nc.scalar.activation(out=h1[m][:], in_=z1[m][:], func=RELU,
                                         bias=bia[:, m:m + 1], scale=1.0)
                u1 = {}
                for kt in range(3):
                    for m in range(2):
                        u = us.tile([128, T], F32R, tag=f"u1_{kt}_{m}",
                                    name=f"u1_{kt}_{m}{sfx}")
                        nc.vector.tensor_scalar(
                            out=u[:], in0=h1[m][:], scalar1=0.0,
                            scalar2=w1r[:, 2 * kt + m:2 * kt + m + 1],
                            op0=GT, op1=MUL)
                        u1[kt, m] = u

                # ---- L2: z2 = W2^T h1 (fp32) ; t2_k = W2^T u1_k (f32r)
                z2 = [ps.tile([128, T], F32, tag="ps", name=f"z2_{m}{sfx}")
                      for m in range(2)]
                for m in range(2):
                    for k in range(2):
                        nc.tensor.matmul(z2[m][:],
                                         lhsT=w2[k][:, bass.ts(m, 128)],
                                         rhs=h1[k][:],
                                         start=(k == 0), stop=(k == 1))
                h2 = [hs.tile([128, T], F32, tag=f"h2_{m}", name=f"h2_{m}{sfx}")
                      for m in range(2)]
                for m in range(2):
                    nc.scalar.activation(out=h2[m][:], in_=z2[m][:], func=RELU,
                                         bias=bia[:, 2 + m:3 + m], scale=1.0)
                u2 = {}
                for kt in range(3):
                    for m in range(2):
                        t2 = ps.tile([128, T], F32, tag="ps",
                                     name=f"t2_{kt}_{m}{sfx}")
                        for k in range(2):
                            nc.tensor.matmul(t2[:],
                                             lhsT=w2r[k][:, bass.ts(m, 128)],
                                             rhs=u1[kt, k][:],
                                             start=(k == 0), stop=(k == 1))
                        u = us.tile([128, T], F32R, tag=f"u2_{kt}_{m}",
                                    name=f"u2_{kt}_{m}{sfx}")
                        nc.vector.scalar_tensor_tensor(
                            out=u[:], in0=h2[m][:], scalar=0.0, in1=t2[:],
                            op0=GT, op1=MUL)
                        u2[kt, m] = u

                # ---- L3: z3 = W3^T h2 (fp32); t3_k = W3^T u2_k (f32r)
                z3 = [ps.tile([128, T], F32, tag="ps", name=f"z3_{m}{sfx}")
                      for m in range(2)]
                for m in range(2):
                    for k in range(2):
                        nc.tensor.matmul(z3[m][:],
                                         lhsT=w3[k][:, bass.ts(m, 128)],
                                         rhs=h2[k][:],
                                         start=(k == 0), stop=(k == 1))
                h3 = [hs.tile([128, T], F32, tag=f"h3_{m}", name=f"h3_{m}{sfx}")
                      for m in range(2)]
                for m in range(2):
                    nc.scalar.activation(out=h3[m][:], in_=z3[m][:], func=RELU,
                                         bias=bia[:, 4 + m:5 + m], scale=1.0)
                u3 = {}
                for kt in range(3):
                    for m in range(2):
                        t3 = ps.tile([128, T], F32, tag="ps",
                                     name=f"t3_{kt}_{m}{sfx}")
                        for k in range(2):
                            nc.tensor.matmul(t3[:],
                                             lhsT=w3r[k][:, bass.ts(m, 128)],
                                             rhs=u2[kt, k][:],
                                             start=(k == 0), stop=(k == 1))
                        u = us.tile([128, T], F32R, tag=f"u3_{kt}_{m}",
                                    name=f"u3_{kt}_{m}{sfx}")
                        nc.vector.scalar_tensor_tensor(
                            out=u[:], in0=h3[m][:], scalar=0.0, in1=t3[:],
                            op0=GT, op1=MUL)
                        u3[kt, m] = u

                # ---- Lout: out[a, t] = sum_{kt,m} u3[kt,m]^T vp[:, 3j:3j+3]
                # j = kt*2 + m maps to V rows [128j : 128j+128]
                op = ps.tile([D, T], F32, tag="ps", name=f"out{sfx}")
                j = 0
                for kt in range(3):
                    for m in range(2):
                        nc.tensor.matmul(op[:],
                                         lhsT=vp[:, 3 * j:3 * (j + 1)],
                                         rhs=u3[kt, m][:],
                                         start=(j == 0), stop=(j == 5))
                        j += 1
                yT = io.tile([D, T], F32, tag="yT", name="yT" + sfx)
                nc.scalar.activation(out=yT[:], in_=op[:], func=COPY,
                                     bias=0.0, scale=1.0)
                nc.scalar.dma_start(out=yt_d[:, t0:t0 + T], in_=yT[:])

            if loop is not None:
                loop.__exit__(None, None, None)

    nc.compile()
    return nc


def _host_prep(W1, b1, W2, b2, W3, b3, Wo, bo):
    """Precompute packed weight tensors shared across cores."""
    W1 = np.asarray(W1, np.float32)
    W2 = np.asarray(W2, np.float32)
    W3 = np.asarray(W3, np.float32)
    Wo = np.asarray(Wo, np.float32)
    # V stack: out = sum_k u3_k @ V_k with
    # V_0 = [0, -Wo0, -Wo1]; V_1 = [Wo0, 0, -Wo2]; V_2 = [Wo1, Wo2, 0]
    V = np.zeros((3, H, 3), np.float32)
    V[0, :, 1] = -Wo[:, 0]
    V[0, :, 2] = -Wo[:, 1]
    V[1, :, 0] = Wo[:, 0]
    V[1, :, 2] = -Wo[:, 2]
    V[2, :, 0] = Wo[:, 1]
    V[2, :, 1] = Wo[:, 2]
    # vp[p, 3j:3j+3] = V_k rows for j = kt*2 + m, row index p of hidden tile m
    vp = np.zeros((128, 18), np.float32)
    for kt in range(3):
        for m in range(2):
            j = kt * 2 + m
            vp[:, 3 * j:3 * (j + 1)] = V[kt, 128 * m:128 * (m + 1), :]
    # w1r[p, 2*kt+m] = W1[kt, 128*m + p]
    w1r = np.zeros((128, 6), np.float32)
    for kt in range(3):
        for m in range(2):
            w1r[:, 2 * kt + m] = W1[kt, 128 * m:128 * (m + 1)]
    # bias[p, 2*l+m] = b_l[128*m + p]
    bias = np.zeros((128, 6), np.float32)
    for li, b in enumerate((b1, b2, b3)):
        b = np.asarray(b, np.float32)
        for m in range(2):
            bias[:, 2 * li + m] = b[128 * m:128 * (m + 1)]
    return dict(w1=W1, w2=W2, w3=W3, vp=vp, w1r=w1r, bias=bias)


_CACHED_NC = None


def kernel(x, W1, b1, W2, b2, W3, b3, Wo, bo):
    global _CACHED_NC
    x = np.ascontiguousarray(np.asarray(x, np.float32))
    assert x.shape == (N, D)
    shared = _host_prep(W1, b1, W2, b2, W3, b3, Wo, bo)
    if _CACHED_NC is None:
        _CACHED_NC = build_nc()
    nc = _CACHED_NC
    in_maps = []
    for c in range(NCORES):
        m = dict(shared)
        m["xt"] = np.ascontiguousarray(x[c * SHARD:(c + 1) * SHARD].T)
        in_maps.append(m)
    res = run_bass_kernel_spmd(nc, in_maps, core_ids=list(range(NCORES)))
    return np.concatenate([r["yt"].T for r in res.results], axis=0)
